# revision 32
# baseline (speedup 1.0000x reference)
"""MultiHeadAttention TRN2 kernel.

Math (B=2, H=16, S=2048, D=128, F=256, DIM=2048):
  Q = einsum('bhsf,hfd', q, Wq) + bq ; K likewise ; V = einsum('bhse,hed', v, Wv) + bv
  P = softmax(Q K^T / 16) ; o = P V ; out = concat_h(o) @ Wo + bo

This environment's cost is dominated by the axon tunnel (~50-90 MB/s host<->
device), not device compute, so the design minimizes wire bytes:
  - the small QKV projections (~4% of FLOPs) run on host BLAS; Q/K/V ship as
    fp16 (rel-err budget is 2e-2; fp16 lands ~4e-4)
  - attention + the Wo product (~96% of FLOPs) run on device in fp16 with
    fp32 PSUM accumulation
  - the per-core Wo partials are summed on device with a single 8-core
    ReduceScatter, and the final 512-row slice leaves as per-row-scaled int8
    (1 MB/core), dequantized on host
  - a persistent jax compilation cache removes the per-call walrus recompile

Sharding: core c -> heads [2c, 2c+2), BOTH batches resident (so each Wo row
block is uploaded once instead of once per batch). Device layout per core
(host does the transposes/projections; head-major so core slices are
contiguous):
  qT/kT [2,B,128,2048] (head j, b, d, s)   vB [2,B,16,128,128] (j, b, s-block, s, d)
  wo [2,128,2048] (j, d, n)                partial [B*2048, 2048] (b*s, n)
ReduceScatter over all 8 cores of the [4096, 2048] partial hands core c rows
[512c, 512c+512) of the summed result: cores 0-3 <-> batch 0, 4-7 <-> batch 1.
"""

import sys

import numpy as np

B, H, S, D, F = 2, 16, 2048, 128, 256
DIM = H * D
NC = 8
HPC = 2  # heads per core (both batches resident)
SC512 = S // 512  # 4
NKT = S // 128  # 16

_BUILT = None
_SCR = None
TRACE = False
LAST_RESULTS = None


def _import_concourse():
    try:
        import concourse.bass  # noqa: F401
    except ImportError:
        sys.path.insert(0, "/opt/trn_rl_repo")
    try:
        import jax

        jax.config.update("jax_compilation_cache_dir", "/tmp/jaxcache")
        jax.config.update("jax_persistent_cache_min_compile_time_secs", 0.0)
        jax.config.update("jax_persistent_cache_min_entry_size_bytes", 0)
    except Exception:
        pass


def _build():
    _import_concourse()
    from contextlib import ExitStack

    import concourse.bass as bass
    import concourse.mybir as mybir
    import concourse.tile as tile

    f32 = mybir.dt.float32
    F16 = mybir.dt.float16
    AF = mybir.ActivationFunctionType

    nc = bass.Bass(target_bir_lowering=False)

    # The axon tunnel charges ~90ms of fixed overhead PER ARRAY on top of
    # bandwidth, so all inputs ride in ONE tensor of [128, 2048] fp16
    # panels: 0-3 QT (j,b), 4-7 KT (j,b), 8-11 V (j,b, tile-ready
    # tok-major), 12-13 Wo (j). Likewise the output is ONE int8 tensor:
    # cols 0:2048 = per-row-scaled int8 values, cols 2048:2052 = the f32
    # row scale bitcast into 4 bytes.
    pack_d = nc.dram_tensor("pack", [14, 128, S], F16, kind="ExternalInput")
    i8 = mybir.dt.int8
    # fp16 partials (both batches stacked) to an internal bounce,
    # ReduceScatter over all 8 cores, then the 512-row slice leaves as
    # per-row-scaled int8 to halve the download (collectives can't touch
    # I/O tensors directly). The HW f32->int8 cast rounds to nearest
    # (verified on device; CoreSim floors -- trust HW), so a plain
    # multiply gives ideal symmetric quantization.
    out_d = nc.dram_tensor("out_p", [B * S, DIM], F16)
    rs_b = nc.dram_tensor("rs_b", [B * S // NC, DIM], F16)
    out_pk = nc.dram_tensor("out_pk", [B * S // NC, DIM + 4], i8,
                            kind="ExternalOutput")

    with ExitStack() as ctx:
        tc = ctx.enter_context(tile.TileContext(nc))
        consts = ctx.enter_context(tc.tile_pool(name="consts", bufs=1))
        big = ctx.enter_context(tc.tile_pool(name="big", bufs=2))
        otn_pool = ctx.enter_context(tc.tile_pool(name="otn", bufs=4))
        sm = ctx.enter_context(tc.tile_pool(name="sm", bufs=2))
        wop = ctx.enter_context(tc.tile_pool(name="wop", bufs=8))
        ps = ctx.enter_context(tc.tile_pool(name="ps", bufs=1, space="PSUM"))

        # ---- constants -------------------------------------------------
        ones_full = consts.tile([128, 128], F16)
        nc.vector.memset(ones_full, 1.0)

        wo_sb = {}

        # ---- P3 group emitter (interleaved into P2 slack + tail) -------
        store_q = [nc.gpsimd, nc.sync, nc.scalar]
        p3_state = {"n": 0}
        p3_pending = []

        def emit_p3_group(b, dc, sc, tail):
            csl = slice(sc * 128, (sc + 1) * 128)
            rsl = slice(b * S + sc * 128, b * S + (sc + 1) * 128)
            dsl = slice(dc * 512, (dc + 1) * 512)
            pw = ps.tile([128, 512], f32, tag="w", bufs=2, name=f"pw{b}_{dc}_{sc}")
            for j in range(HPC):
                nc.tensor.matmul(pw, otn[b, j][:, csl], wo_sb[dc, j],
                                 start=(j == 0), stop=(j == HPC - 1))
            ow = sm.tile([128, 512], F16, tag="ow", bufs=3, name=f"ow{b}_{dc}_{sc}")
            # during interleave keep drains off ACT (the bottleneck engine)
            if tail and p3_state["n"] % 2 == 0:
                nc.scalar.copy(out=ow, in_=pw)
            else:
                nc.vector.tensor_copy(out=ow, in_=pw)
            store_q[p3_state["n"] % 3].dma_start(out=out_d[rsl, dsl], in_=ow)
            p3_state["n"] += 1

        otn = {}

        # ---- P1: load unit u's projected Q/K/V (host did the matmuls) --
        def emit_head_dmas(u):
            b, j = u // HPC, u % HPC
            p = j * B + b
            QT = big.tile([128, S], F16, tag="QT", name=f"QT{u}")
            nc.sync.dma_start(out=QT, in_=pack_d[p])
            KT = big.tile([128, S], F16, tag="KT", name=f"KT{u}")
            nc.gpsimd.dma_start(out=KT, in_=pack_d[4 + p])
            Vsb = big.tile([128, S], F16, tag="V", name=f"V{u}")
            nc.scalar.dma_start(out=Vsb, in_=pack_d[8 + p])
            return QT, KT, Vsb

        hd = emit_head_dmas(0)
        for dc in range(DIM // 512):
            for j in range(HPC):
                w = wop.tile([128, 512], F16, tag="wo", bufs=8,
                             name=f"wo{dc}_{j}")
                nc.scalar.dma_start(out=w,
                                    in_=pack_d[12 + j, :, dc * 512 : (dc + 1) * 512])
                wo_sb[dc, j] = w
        for u in range(B * HPC):
            b, j = u // HPC, u % HPC
            QT, KT, Vsb = hd
            if u + 1 < B * HPC:
                hd = emit_head_dmas(u + 1)

            # ---- P2: attention for (batch b, head j) -------------------
            oTn = otn_pool.tile([128, S], F16, tag="otn", name=f"oTn{u}")
            otn[b, j] = oTn
            for qc in range(SC512):
                qsl = slice(qc * 512, (qc + 1) * 512)
                po = ps.tile([128, 512], f32, tag="o", bufs=2, name=f"po{u}_{qc}")
                pr = ps.tile([128, 512], f32, tag="r", bufs=1, name=f"pr{u}_{qc}")

                def emit_pscore(kt):
                    csl = slice(kt * 128, (kt + 1) * 128)
                    t = ps.tile([128, 512], f32, tag="s", bufs=3,
                                name=f"ps{u}_{qc}_{kt}")
                    nc.tensor.matmul(t, KT[:, csl], QT[:, qsl],
                                     start=True, stop=True)
                    return t

                # software pipeline: pscore(kt+1) is emitted before po(kt)
                # so PE's in-order queue keeps ACT fed with score tiles
                # while po waits on exp(kt); otherwise every exp gets a
                # PE->ACT round-trip bubble on the bottleneck engine
                cur = emit_pscore(0)
                for kt in range(NKT):
                    csl = slice(kt * 128, (kt + 1) * 128)
                    pT = sm.tile([128, 512], F16, tag="pT", bufs=3,
                                 name=f"pT{u}_{qc}_{kt}")
                    nc.scalar.activation(out=pT, in_=cur, func=AF.Exp,
                                         bias=0.0, scale=0.0625)
                    if kt + 1 < NKT:
                        cur = emit_pscore(kt + 1)
                    nc.tensor.matmul(po, Vsb[:, csl], pT,
                                     start=(kt == 0), stop=(kt == NKT - 1))
                    nc.tensor.matmul(pr, ones_full, pT,
                                     start=(kt == 0), stop=(kt == NKT - 1))
                    # PE slack under the ACT exp bottleneck: fold one output
                    # projection group per kt slot once its tokens are done
                    if p3_pending:
                        emit_p3_group(*p3_pending.pop(0), tail=False)
                rr = sm.tile([128, 512], f32, tag="rr_sb", bufs=2, name=f"rr{u}_{qc}")
                nc.vector.reciprocal(out=rr, in_=pr)
                nc.vector.tensor_mul(out=oTn[:, qsl], in0=po, in1=rr)
                if j == HPC - 1:
                    # batch b's heads are both done for this qc's tokens
                    p3_pending.extend(
                        (b, dc, sc)
                        for sc in range(qc * 4, (qc + 1) * 4)
                        for dc in range(DIM // 512))

        # ---- P3 tail: groups not hidden inside P2 ----------------------
        while p3_pending:
            emit_p3_group(*p3_pending.pop(0), tail=True)

        # ---- P4: cross-core reduce of the Wo partials ------------------
        nc.gpsimd.collective_compute(
            "ReduceScatter",
            mybir.AluOpType.add,
            replica_groups=[list(range(NC))],
            ins=[out_d[:].opt()],
            outs=[rs_b[:].opt()],
        )

        # ---- P5: per-row int8 quantization of the reduced slice --------
        for t in range(4):
            rsl = slice(t * 128, (t + 1) * 128)
            x = sm.tile([128, DIM], F16, tag="qx", bufs=2, name=f"qx{t}")
            nc.sync.dma_start(out=x, in_=rs_b[rsl])
            m = sm.tile([128, 1], f32, tag="qm", bufs=2, name=f"qm{t}")
            nc.vector.tensor_reduce(out=m, in_=x, axis=mybir.AxisListType.X,
                                    op=mybir.AluOpType.max,
                                    apply_absolute_value=True)
            nc.vector.tensor_scalar_max(out=m, in0=m, scalar1=1e-6)
            r = sm.tile([128, 1], f32, tag="qr", bufs=2, name=f"qr{t}")
            nc.vector.reciprocal(out=r, in_=m)
            r127 = sm.tile([128, 1], f32, tag="qr7", bufs=2, name=f"qr7{t}")
            nc.vector.tensor_scalar_mul(out=r127, in0=r, scalar1=127.0)
            qt = sm.tile([128, DIM], i8, tag="qq", bufs=2, name=f"qq{t}")
            nc.vector.tensor_scalar_mul(out=qt, in0=x, scalar1=r127)
            nc.sync.dma_start(out=out_pk[rsl, 0:DIM], in_=qt)
            sct = sm.tile([128, 1], f32, tag="qs", bufs=2, name=f"qs{t}")
            nc.vector.tensor_scalar_mul(out=sct, in0=m, scalar1=1.0 / 127.0)
            nc.gpsimd.dma_start(out=out_pk[rsl, DIM : DIM + 4],
                                in_=sct[:, :].bitcast(i8))

    _split_excess_waits(nc)
    return nc


def _split_excess_waits(nc):
    """Compute-engine instructions (Matmult, TensorScalarPtr, ...) only have
    one sync-wait slot in walrus codegen. Split any excess waits onto
    same-engine NoOps inserted just before the instruction."""
    import concourse.mybir as mybir

    n = 0
    for func in nc.m.functions:
        for block in func.blocks:
            out = []
            for inst in block.instructions:
                si = getattr(inst, "sync_info", None)
                if si is not None and si.on_wait and len(si.on_wait) > 1:
                    for w in si.on_wait[:-1]:
                        nop = mybir.InstNoOp(
                            name=f"wsplit_{n}",
                            engine=inst.engine,
                            sync_info=mybir.SyncInfo(on_wait=[w], on_update=[]),
                            bass_nofuse=True,
                        )
                        n += 1
                        out.append(nop)
                    inst.sync_info = mybir.SyncInfo(
                        on_wait=[si.on_wait[-1]], on_update=si.on_update)
                out.append(inst)
            block.instructions[:] = out
    return n


def _scratch():
    global _SCR
    if _SCR is None:
        _SCR = {
            "qf": np.empty((H, B, D, S), np.float32),
            "kf": np.empty((H, B, D, S), np.float32),
            "vf": np.empty((H, B, S, D), np.float32),
            "pack": np.empty((NC, 14, 128, S), np.float16),
        }
    return _SCR


def kernel(q, k, v, Wq, Wk, Wv, bq, bk, bv, Wo, bo):
    global _BUILT, LAST_RESULTS
    _import_concourse()
    from concourse.bass_utils import run_bass_kernel_spmd

    q = np.asarray(q, dtype=np.float32)
    k = np.asarray(k, dtype=np.float32)
    v = np.asarray(v, dtype=np.float32)
    Wq = np.asarray(Wq, dtype=np.float32)
    Wk = np.asarray(Wk, dtype=np.float32)
    Wv = np.asarray(Wv, dtype=np.float32)
    bq = np.asarray(bq, dtype=np.float32)
    bk = np.asarray(bk, dtype=np.float32)
    bv = np.asarray(bv, dtype=np.float32)
    Wo = np.asarray(Wo, dtype=np.float32)
    bo = np.asarray(bo, dtype=np.float32)

    # host QKV projections (fp32 BLAS into reused scratch, then fp16 bias-
    # add+cast straight into each core's panel of the single packed wire
    # tensor; per-core slices are contiguous)
    s = _scratch()
    np.matmul(Wq.transpose(0, 2, 1)[:, None], q.transpose(1, 0, 3, 2), out=s["qf"])
    np.matmul(Wk.transpose(0, 2, 1)[:, None], k.transpose(1, 0, 3, 2), out=s["kf"])
    np.matmul(v.transpose(1, 0, 2, 3), Wv[:, None], out=s["vf"])
    qf = s["qf"].reshape(NC, HPC, B, 128, S)
    kf = s["kf"].reshape(NC, HPC, B, 128, S)
    vf = s["vf"].reshape(NC, HPC, B, NKT, 128, D)
    bq5 = bq.reshape(NC, HPC, 1, 128, 1)
    bk5 = bk.reshape(NC, HPC, 1, 128, 1)
    bv5 = bv.reshape(NC, HPC, 1, 1, 1, D)
    Wo3 = Wo.reshape(H, D, DIM)
    pk = s["pack"]
    for c in range(NC):
        np.add(qf[c], bq5[c], out=pk[c, 0:4].reshape(HPC, B, 128, S))
        np.add(kf[c], bk5[c], out=pk[c, 4:8].reshape(HPC, B, 128, S))
        vv = pk[c, 8:12].reshape(HPC, B, 128, NKT, D).transpose(0, 1, 3, 2, 4)
        np.add(vf[c], bv5[c], out=vv)
        pk[c, 12:14] = Wo3[c * HPC : (c + 1) * HPC]

    if _BUILT is None:
        _BUILT = _build()
    in_maps = [{"pack": pk[c]} for c in range(NC)]
    res = run_bass_kernel_spmd(_BUILT, in_maps, core_ids=list(range(NC)),
                               trace=TRACE)
    LAST_RESULTS = res
    out = np.empty((B, S, DIM), dtype=np.float32)
    for c in range(NC):
        b, r = c // 4, c % 4
        sl = out[b, r * 512 : (r + 1) * 512]
        arr = res.results[c]["out_pk"]
        sc = np.ascontiguousarray(arr[:, DIM : DIM + 4]).view(np.float32)
        np.multiply(arr[:, 0:DIM], sc, out=sl)
        sl += bo
    return out


# revision 33
# speedup vs baseline: 1.5463x; 1.5463x over previous
"""MultiHeadAttention TRN2 kernel.

Math (B=2, H=16, S=2048, D=128, F=256, DIM=2048):
  Q = einsum('bhsf,hfd', q, Wq) + bq ; K likewise ; V = einsum('bhse,hed', v, Wv) + bv
  P = softmax(Q K^T / 16) ; o = P V ; out = concat_h(o) @ Wo + bo

This environment's cost is dominated by the axon tunnel (~50-90 MB/s host<->
device), not device compute, so the design minimizes wire bytes:
  - the small QKV projections (~4% of FLOPs) run on host BLAS; Q/K/V ship as
    fp16 (rel-err budget is 2e-2; fp16 lands ~4e-4)
  - attention + the Wo product (~96% of FLOPs) run on device in fp16 with
    fp32 PSUM accumulation
  - the per-core Wo partials are summed on device with a single 8-core
    ReduceScatter, and the final 512-row slice leaves as per-row-scaled int8
    (1 MB/core), dequantized on host
  - a persistent jax compilation cache removes the per-call walrus recompile

Sharding: core c -> heads [2c, 2c+2), BOTH batches resident (so each Wo row
block is uploaded once instead of once per batch). Device layout per core
(host does the transposes/projections; head-major so core slices are
contiguous):
  qT/kT [2,B,128,2048] (head j, b, d, s)   vB [2,B,16,128,128] (j, b, s-block, s, d)
  wo [2,128,2048] (j, d, n)                partial [B*2048, 2048] (b*s, n)
ReduceScatter over all 8 cores of the [4096, 2048] partial hands core c rows
[512c, 512c+512) of the summed result: cores 0-3 <-> batch 0, 4-7 <-> batch 1.
"""

import sys

import numpy as np

B, H, S, D, F = 2, 16, 2048, 128, 256
DIM = H * D
NC = 8
HPC = 2  # heads per core (both batches resident)
SC512 = S // 512  # 4
NKT = S // 128  # 16

_BUILT = None
_SCR = None
TRACE = False
LAST_RESULTS = None


def _import_concourse():
    try:
        import concourse.bass  # noqa: F401
    except ImportError:
        sys.path.insert(0, "/opt/trn_rl_repo")
    try:
        import jax

        jax.config.update("jax_compilation_cache_dir", "/tmp/jaxcache")
        jax.config.update("jax_persistent_cache_min_compile_time_secs", 0.0)
        jax.config.update("jax_persistent_cache_min_entry_size_bytes", 0)
    except Exception:
        pass


def _build():
    _import_concourse()
    from contextlib import ExitStack

    import concourse.bass as bass
    import concourse.mybir as mybir
    import concourse.tile as tile

    f32 = mybir.dt.float32
    F16 = mybir.dt.float16
    AF = mybir.ActivationFunctionType

    nc = bass.Bass(target_bir_lowering=False)

    # head-major layouts so each core's slice of the host arrays is
    # contiguous (cheap concat on the way to the devices)
    qT_d = nc.dram_tensor("qT", [HPC, B, 128, S], F16, kind="ExternalInput")
    kT_d = nc.dram_tensor("kT", [HPC, B, 128, S], F16, kind="ExternalInput")
    vB_d = nc.dram_tensor("vB", [HPC, B, NKT, 128, 128], F16, kind="ExternalInput")
    wo_d = nc.dram_tensor("wo", [HPC, 128, DIM], F16, kind="ExternalInput")
    i8 = mybir.dt.int8
    # fp16 partials (both batches stacked) to an internal bounce,
    # ReduceScatter over all 8 cores, then the 512-row slice leaves as
    # per-row-scaled int8 to halve the download (collectives can't touch
    # I/O tensors directly). The HW f32->int8 cast rounds to nearest
    # (verified on device; CoreSim floors -- trust HW), so a plain
    # multiply gives ideal symmetric quantization.
    out_d = nc.dram_tensor("out_p", [B * S, DIM], F16)
    rs_b = nc.dram_tensor("rs_b", [B * S // NC, DIM], F16)
    out_q = nc.dram_tensor("out_q", [B * S // NC, DIM], i8, kind="ExternalOutput")
    out_sc = nc.dram_tensor("out_sc", [B * S // NC, 1], f32, kind="ExternalOutput")

    with ExitStack() as ctx:
        tc = ctx.enter_context(tile.TileContext(nc))
        consts = ctx.enter_context(tc.tile_pool(name="consts", bufs=1))
        big = ctx.enter_context(tc.tile_pool(name="big", bufs=2))
        otn_pool = ctx.enter_context(tc.tile_pool(name="otn", bufs=4))
        sm = ctx.enter_context(tc.tile_pool(name="sm", bufs=2))
        wop = ctx.enter_context(tc.tile_pool(name="wop", bufs=8))
        ps = ctx.enter_context(tc.tile_pool(name="ps", bufs=1, space="PSUM"))

        # ---- constants -------------------------------------------------
        ones_full = consts.tile([128, 128], F16)
        nc.vector.memset(ones_full, 1.0)

        wo_sb = {}

        # ---- P3 group emitter (interleaved into P2 slack + tail) -------
        store_q = [nc.gpsimd, nc.sync, nc.scalar]
        p3_state = {"n": 0}
        p3_pending = []

        def emit_p3_group(b, dc, sc, tail):
            csl = slice(sc * 128, (sc + 1) * 128)
            rsl = slice(b * S + sc * 128, b * S + (sc + 1) * 128)
            dsl = slice(dc * 512, (dc + 1) * 512)
            pw = ps.tile([128, 512], f32, tag="w", bufs=2, name=f"pw{b}_{dc}_{sc}")
            for j in range(HPC):
                nc.tensor.matmul(pw, otn[b, j][:, csl], wo_sb[dc, j],
                                 start=(j == 0), stop=(j == HPC - 1))
            ow = sm.tile([128, 512], F16, tag="ow", bufs=3, name=f"ow{b}_{dc}_{sc}")
            # during interleave keep drains off ACT (the bottleneck engine)
            if tail and p3_state["n"] % 2 == 0:
                nc.scalar.copy(out=ow, in_=pw)
            else:
                nc.vector.tensor_copy(out=ow, in_=pw)
            store_q[p3_state["n"] % 3].dma_start(out=out_d[rsl, dsl], in_=ow)
            p3_state["n"] += 1

        otn = {}

        # ---- P1: load unit u's projected Q/K/V (host did the matmuls) --
        def emit_head_dmas(u):
            b, j = u // HPC, u % HPC
            QT = big.tile([128, S], F16, tag="QT", name=f"QT{u}")
            nc.sync.dma_start(out=QT, in_=qT_d[j, b])
            KT = big.tile([128, S], F16, tag="KT", name=f"KT{u}")
            nc.gpsimd.dma_start(out=KT, in_=kT_d[j, b])
            Vsb = big.tile([128, S], F16, tag="V", name=f"V{u}")
            for kt in range(NKT):
                csl = slice(kt * 128, (kt + 1) * 128)
                (nc.scalar if kt % 2 == 0 else nc.sync).dma_start(
                    out=Vsb[:, csl], in_=vB_d[j, b, kt])
            return QT, KT, Vsb

        hd = emit_head_dmas(0)
        for dc in range(DIM // 512):
            for j in range(HPC):
                w = wop.tile([128, 512], F16, tag="wo", bufs=8,
                             name=f"wo{dc}_{j}")
                nc.scalar.dma_start(out=w, in_=wo_d[j, :, dc * 512 : (dc + 1) * 512])
                wo_sb[dc, j] = w
        for u in range(B * HPC):
            b, j = u // HPC, u % HPC
            QT, KT, Vsb = hd
            if u + 1 < B * HPC:
                hd = emit_head_dmas(u + 1)

            # ---- P2: attention for (batch b, head j) -------------------
            oTn = otn_pool.tile([128, S], F16, tag="otn", name=f"oTn{u}")
            otn[b, j] = oTn
            for qc in range(SC512):
                qsl = slice(qc * 512, (qc + 1) * 512)
                po = ps.tile([128, 512], f32, tag="o", bufs=2, name=f"po{u}_{qc}")
                pr = ps.tile([128, 512], f32, tag="r", bufs=1, name=f"pr{u}_{qc}")

                def emit_pscore(kt):
                    csl = slice(kt * 128, (kt + 1) * 128)
                    t = ps.tile([128, 512], f32, tag="s", bufs=3,
                                name=f"ps{u}_{qc}_{kt}")
                    nc.tensor.matmul(t, KT[:, csl], QT[:, qsl],
                                     start=True, stop=True)
                    return t

                # software pipeline: pscore(kt+1) is emitted before po(kt)
                # so PE's in-order queue keeps ACT fed with score tiles
                # while po waits on exp(kt); otherwise every exp gets a
                # PE->ACT round-trip bubble on the bottleneck engine
                cur = emit_pscore(0)
                for kt in range(NKT):
                    csl = slice(kt * 128, (kt + 1) * 128)
                    pT = sm.tile([128, 512], F16, tag="pT", bufs=3,
                                 name=f"pT{u}_{qc}_{kt}")
                    nc.scalar.activation(out=pT, in_=cur, func=AF.Exp,
                                         bias=0.0, scale=0.0625)
                    if kt + 1 < NKT:
                        cur = emit_pscore(kt + 1)
                    nc.tensor.matmul(po, Vsb[:, csl], pT,
                                     start=(kt == 0), stop=(kt == NKT - 1))
                    nc.tensor.matmul(pr, ones_full, pT,
                                     start=(kt == 0), stop=(kt == NKT - 1))
                    # PE slack under the ACT exp bottleneck: fold one output
                    # projection group per kt slot once its tokens are done
                    if p3_pending:
                        emit_p3_group(*p3_pending.pop(0), tail=False)
                rr = sm.tile([128, 512], f32, tag="rr_sb", bufs=2, name=f"rr{u}_{qc}")
                nc.vector.reciprocal(out=rr, in_=pr)
                nc.vector.tensor_mul(out=oTn[:, qsl], in0=po, in1=rr)
                if j == HPC - 1:
                    # batch b's heads are both done for this qc's tokens
                    p3_pending.extend(
                        (b, dc, sc)
                        for sc in range(qc * 4, (qc + 1) * 4)
                        for dc in range(DIM // 512))

        # ---- P3 tail: groups not hidden inside P2 ----------------------
        while p3_pending:
            emit_p3_group(*p3_pending.pop(0), tail=True)

        # ---- P4: cross-core reduce of the Wo partials ------------------
        nc.gpsimd.collective_compute(
            "ReduceScatter",
            mybir.AluOpType.add,
            replica_groups=[list(range(NC))],
            ins=[out_d[:].opt()],
            outs=[rs_b[:].opt()],
        )

        # ---- P5: per-row int8 quantization of the reduced slice --------
        for t in range(4):
            rsl = slice(t * 128, (t + 1) * 128)
            x = sm.tile([128, DIM], F16, tag="qx", bufs=2, name=f"qx{t}")
            nc.sync.dma_start(out=x, in_=rs_b[rsl])
            m = sm.tile([128, 1], f32, tag="qm", bufs=2, name=f"qm{t}")
            nc.vector.tensor_reduce(out=m, in_=x, axis=mybir.AxisListType.X,
                                    op=mybir.AluOpType.max,
                                    apply_absolute_value=True)
            nc.vector.tensor_scalar_max(out=m, in0=m, scalar1=1e-6)
            r = sm.tile([128, 1], f32, tag="qr", bufs=2, name=f"qr{t}")
            nc.vector.reciprocal(out=r, in_=m)
            r127 = sm.tile([128, 1], f32, tag="qr7", bufs=2, name=f"qr7{t}")
            nc.vector.tensor_scalar_mul(out=r127, in0=r, scalar1=127.0)
            qt = sm.tile([128, DIM], i8, tag="qq", bufs=2, name=f"qq{t}")
            nc.vector.tensor_scalar_mul(out=qt, in0=x, scalar1=r127)
            nc.sync.dma_start(out=out_q[rsl], in_=qt)
            sct = sm.tile([128, 1], f32, tag="qs", bufs=2, name=f"qs{t}")
            nc.vector.tensor_scalar_mul(out=sct, in0=m, scalar1=1.0 / 127.0)
            nc.gpsimd.dma_start(out=out_sc[rsl], in_=sct)

    _split_excess_waits(nc)
    return nc


def _split_excess_waits(nc):
    """Compute-engine instructions (Matmult, TensorScalarPtr, ...) only have
    one sync-wait slot in walrus codegen. Split any excess waits onto
    same-engine NoOps inserted just before the instruction."""
    import concourse.mybir as mybir

    n = 0
    for func in nc.m.functions:
        for block in func.blocks:
            out = []
            for inst in block.instructions:
                si = getattr(inst, "sync_info", None)
                if si is not None and si.on_wait and len(si.on_wait) > 1:
                    for w in si.on_wait[:-1]:
                        nop = mybir.InstNoOp(
                            name=f"wsplit_{n}",
                            engine=inst.engine,
                            sync_info=mybir.SyncInfo(on_wait=[w], on_update=[]),
                            bass_nofuse=True,
                        )
                        n += 1
                        out.append(nop)
                    inst.sync_info = mybir.SyncInfo(
                        on_wait=[si.on_wait[-1]], on_update=si.on_update)
                out.append(inst)
            block.instructions[:] = out
    return n


def _scratch():
    global _SCR
    if _SCR is None:
        _SCR = {
            "qf": np.empty((H, B, D, S), np.float32),
            "kf": np.empty((H, B, D, S), np.float32),
            "vf": np.empty((H, B, S, D), np.float32),
            "qh": np.empty((H, B, D, S), np.float16),
            "kh": np.empty((H, B, D, S), np.float16),
            "vh": np.empty((H, B, S, D), np.float16),
            "wo": np.empty((H, D, DIM), np.float16),
        }
    return _SCR


def kernel(q, k, v, Wq, Wk, Wv, bq, bk, bv, Wo, bo):
    global _BUILT, LAST_RESULTS
    _import_concourse()
    from concourse.bass_utils import run_bass_kernel_spmd

    q = np.asarray(q, dtype=np.float32)
    k = np.asarray(k, dtype=np.float32)
    v = np.asarray(v, dtype=np.float32)
    Wq = np.asarray(Wq, dtype=np.float32)
    Wk = np.asarray(Wk, dtype=np.float32)
    Wv = np.asarray(Wv, dtype=np.float32)
    bq = np.asarray(bq, dtype=np.float32)
    bk = np.asarray(bk, dtype=np.float32)
    bv = np.asarray(bv, dtype=np.float32)
    Wo = np.asarray(Wo, dtype=np.float32)
    bo = np.asarray(bo, dtype=np.float32)

    # host QKV projections (fp32 BLAS into reused scratch, then fp16 wire;
    # head-major so per-core slices below are contiguous views)
    s = _scratch()
    np.matmul(Wq.transpose(0, 2, 1)[:, None], q.transpose(1, 0, 3, 2), out=s["qf"])
    np.add(s["qf"], bq[:, None, :, None], out=s["qh"])       # [H,B,D,S] f16
    np.matmul(Wk.transpose(0, 2, 1)[:, None], k.transpose(1, 0, 3, 2), out=s["kf"])
    np.add(s["kf"], bk[:, None, :, None], out=s["kh"])       # [H,B,D,S] f16
    np.matmul(v.transpose(1, 0, 2, 3), Wv[:, None], out=s["vf"])
    np.add(s["vf"], bv[:, None, None, :], out=s["vh"])       # [H,B,S,D] f16
    s["wo"][...] = Wo.reshape(H, D, DIM)
    VB16 = s["vh"].reshape(H, B, NKT, 128, D)

    if _BUILT is None:
        _BUILT = _build()
    in_maps = []
    for c in range(NC):
        hs = slice(c * HPC, (c + 1) * HPC)
        in_maps.append({"qT": s["qh"][hs], "kT": s["kh"][hs],
                        "vB": VB16[hs], "wo": s["wo"][hs]})
    res = run_bass_kernel_spmd(_BUILT, in_maps, core_ids=list(range(NC)),
                               trace=TRACE)
    LAST_RESULTS = res
    out = np.empty((B, S, DIM), dtype=np.float32)
    for c in range(NC):
        b, r = c // 4, c % 4
        sl = out[b, r * 512 : (r + 1) * 512]
        np.multiply(res.results[c]["out_q"], res.results[c]["out_sc"], out=sl)
        sl += bo
    return out


# revision 35
# speedup vs baseline: 1.5916x; 1.0293x over previous
"""MultiHeadAttention TRN2 kernel.

Math (B=2, H=16, S=2048, D=128, F=256, DIM=2048):
  Q = einsum('bhsf,hfd', q, Wq) + bq ; K likewise ; V = einsum('bhse,hed', v, Wv) + bv
  P = softmax(Q K^T / 16) ; o = P V ; out = concat_h(o) @ Wo + bo

This environment's cost is dominated by the axon tunnel (~50-90 MB/s host<->
device), not device compute, so the design minimizes wire bytes:
  - the small QKV projections (~4% of FLOPs) run on host BLAS; Q/K/V ship as
    fp16 (rel-err budget is 2e-2; fp16 lands ~4e-4)
  - attention + the Wo product (~96% of FLOPs) run on device in fp16 with
    fp32 PSUM accumulation
  - the per-core Wo partials are summed on device with a single 8-core
    ReduceScatter, and the final 512-row slice leaves as per-row-scaled int8
    (1 MB/core), dequantized on host
  - a persistent jax compilation cache removes the per-call walrus recompile

Sharding: core c -> heads [2c, 2c+2), BOTH batches resident (so each Wo row
block is uploaded once instead of once per batch). Device layout per core
(host does the transposes/projections; head-major so core slices are
contiguous):
  qT/kT [2,B,128,2048] (head j, b, d, s)   vB [2,B,16,128,128] (j, b, s-block, s, d)
  wo [2,128,2048] (j, d, n)                partial [B*2048, 2048] (b*s, n)
ReduceScatter over all 8 cores of the [4096, 2048] partial hands core c rows
[512c, 512c+512) of the summed result: cores 0-3 <-> batch 0, 4-7 <-> batch 1.
"""

import sys
import time

import numpy as np

B, H, S, D, F = 2, 16, 2048, 128, 256
DIM = H * D
NC = 8
HPC = 2  # heads per core (both batches resident)
SC512 = S // 512  # 4
NKT = S // 128  # 16

_BUILT = None
_SCR = None
TRACE = False
LAST_RESULTS = None


def _import_concourse():
    try:
        import concourse.bass  # noqa: F401
    except ImportError:
        sys.path.insert(0, "/opt/trn_rl_repo")
    try:
        import jax

        jax.config.update("jax_compilation_cache_dir", "/tmp/jaxcache")
        jax.config.update("jax_persistent_cache_min_compile_time_secs", 0.0)
        jax.config.update("jax_persistent_cache_min_entry_size_bytes", 0)
    except Exception:
        pass


def _build():
    _import_concourse()
    from contextlib import ExitStack

    import concourse.bass as bass
    import concourse.mybir as mybir
    import concourse.tile as tile

    f32 = mybir.dt.float32
    F16 = mybir.dt.float16
    AF = mybir.ActivationFunctionType

    nc = bass.Bass(target_bir_lowering=False)

    # head-major layouts so each core's slice of the host arrays is
    # contiguous (cheap concat on the way to the devices)
    qT_d = nc.dram_tensor("qT", [HPC, B, 128, S], F16, kind="ExternalInput")
    kT_d = nc.dram_tensor("kT", [HPC, B, 128, S], F16, kind="ExternalInput")
    vB_d = nc.dram_tensor("vB", [HPC, B, NKT, 128, 128], F16, kind="ExternalInput")
    wo_d = nc.dram_tensor("wo", [HPC, 128, DIM], F16, kind="ExternalInput")
    i8 = mybir.dt.int8
    # fp16 partials (both batches stacked) to an internal bounce,
    # ReduceScatter over all 8 cores, then the 512-row slice leaves as
    # per-row-scaled int8 to halve the download (collectives can't touch
    # I/O tensors directly). The HW f32->int8 cast rounds to nearest
    # (verified on device; CoreSim floors -- trust HW), so a plain
    # multiply gives ideal symmetric quantization.
    out_d = nc.dram_tensor("out_p", [B * S, DIM], F16)
    rs_b = nc.dram_tensor("rs_b", [B * S // NC, DIM], F16)
    out_q = nc.dram_tensor("out_q", [B * S // NC, DIM], i8, kind="ExternalOutput")
    out_sc = nc.dram_tensor("out_sc", [B * S // NC, 1], f32, kind="ExternalOutput")

    with ExitStack() as ctx:
        tc = ctx.enter_context(tile.TileContext(nc))
        consts = ctx.enter_context(tc.tile_pool(name="consts", bufs=1))
        big = ctx.enter_context(tc.tile_pool(name="big", bufs=2))
        otn_pool = ctx.enter_context(tc.tile_pool(name="otn", bufs=4))
        sm = ctx.enter_context(tc.tile_pool(name="sm", bufs=2))
        wop = ctx.enter_context(tc.tile_pool(name="wop", bufs=8))
        ps = ctx.enter_context(tc.tile_pool(name="ps", bufs=1, space="PSUM"))

        # ---- constants -------------------------------------------------
        ones_full = consts.tile([128, 128], F16)
        nc.vector.memset(ones_full, 1.0)

        wo_sb = {}

        # ---- P3 group emitter (interleaved into P2 slack + tail) -------
        store_q = [nc.gpsimd, nc.sync, nc.scalar]
        p3_state = {"n": 0}
        p3_pending = []

        def emit_p3_group(b, dc, sc, tail):
            csl = slice(sc * 128, (sc + 1) * 128)
            rsl = slice(b * S + sc * 128, b * S + (sc + 1) * 128)
            dsl = slice(dc * 512, (dc + 1) * 512)
            pw = ps.tile([128, 512], f32, tag="w", bufs=2, name=f"pw{b}_{dc}_{sc}")
            for j in range(HPC):
                nc.tensor.matmul(pw, otn[b, j][:, csl], wo_sb[dc, j],
                                 start=(j == 0), stop=(j == HPC - 1))
            ow = sm.tile([128, 512], F16, tag="ow", bufs=3, name=f"ow{b}_{dc}_{sc}")
            # during interleave keep drains off ACT (the bottleneck engine)
            if tail and p3_state["n"] % 2 == 0:
                nc.scalar.copy(out=ow, in_=pw)
            else:
                nc.vector.tensor_copy(out=ow, in_=pw)
            store_q[p3_state["n"] % 3].dma_start(out=out_d[rsl, dsl], in_=ow)
            p3_state["n"] += 1

        otn = {}

        # ---- P1: load unit u's projected Q/K/V (host did the matmuls) --
        def emit_head_dmas(u):
            b, j = u // HPC, u % HPC
            QT = big.tile([128, S], F16, tag="QT", name=f"QT{u}")
            nc.sync.dma_start(out=QT, in_=qT_d[j, b])
            KT = big.tile([128, S], F16, tag="KT", name=f"KT{u}")
            nc.gpsimd.dma_start(out=KT, in_=kT_d[j, b])
            Vsb = big.tile([128, S], F16, tag="V", name=f"V{u}")
            for kt in range(NKT):
                csl = slice(kt * 128, (kt + 1) * 128)
                (nc.scalar if kt % 2 == 0 else nc.sync).dma_start(
                    out=Vsb[:, csl], in_=vB_d[j, b, kt])
            return QT, KT, Vsb

        hd = emit_head_dmas(0)
        for dc in range(DIM // 512):
            for j in range(HPC):
                w = wop.tile([128, 512], F16, tag="wo", bufs=8,
                             name=f"wo{dc}_{j}")
                nc.scalar.dma_start(out=w, in_=wo_d[j, :, dc * 512 : (dc + 1) * 512])
                wo_sb[dc, j] = w
        for u in range(B * HPC):
            b, j = u // HPC, u % HPC
            QT, KT, Vsb = hd
            if u + 1 < B * HPC:
                hd = emit_head_dmas(u + 1)

            # ---- P2: attention for (batch b, head j) -------------------
            oTn = otn_pool.tile([128, S], F16, tag="otn", name=f"oTn{u}")
            otn[b, j] = oTn
            for qc in range(SC512):
                qsl = slice(qc * 512, (qc + 1) * 512)
                po = ps.tile([128, 512], f32, tag="o", bufs=2, name=f"po{u}_{qc}")
                pr = ps.tile([128, 512], f32, tag="r", bufs=1, name=f"pr{u}_{qc}")

                def emit_pscore(kt):
                    csl = slice(kt * 128, (kt + 1) * 128)
                    t = ps.tile([128, 512], f32, tag="s", bufs=3,
                                name=f"ps{u}_{qc}_{kt}")
                    nc.tensor.matmul(t, KT[:, csl], QT[:, qsl],
                                     start=True, stop=True)
                    return t

                # software pipeline: pscore(kt+1) is emitted before po(kt)
                # so PE's in-order queue keeps ACT fed with score tiles
                # while po waits on exp(kt); otherwise every exp gets a
                # PE->ACT round-trip bubble on the bottleneck engine
                cur = emit_pscore(0)
                for kt in range(NKT):
                    csl = slice(kt * 128, (kt + 1) * 128)
                    pT = sm.tile([128, 512], F16, tag="pT", bufs=3,
                                 name=f"pT{u}_{qc}_{kt}")
                    nc.scalar.activation(out=pT, in_=cur, func=AF.Exp,
                                         bias=0.0, scale=0.0625)
                    if kt + 1 < NKT:
                        cur = emit_pscore(kt + 1)
                    nc.tensor.matmul(po, Vsb[:, csl], pT,
                                     start=(kt == 0), stop=(kt == NKT - 1))
                    nc.tensor.matmul(pr, ones_full, pT,
                                     start=(kt == 0), stop=(kt == NKT - 1))
                    # PE slack under the ACT exp bottleneck: fold one output
                    # projection group per kt slot once its tokens are done
                    if p3_pending:
                        emit_p3_group(*p3_pending.pop(0), tail=False)
                rr = sm.tile([128, 512], f32, tag="rr_sb", bufs=2, name=f"rr{u}_{qc}")
                nc.vector.reciprocal(out=rr, in_=pr)
                nc.vector.tensor_mul(out=oTn[:, qsl], in0=po, in1=rr)
                if j == HPC - 1:
                    # batch b's heads are both done for this qc's tokens
                    p3_pending.extend(
                        (b, dc, sc)
                        for sc in range(qc * 4, (qc + 1) * 4)
                        for dc in range(DIM // 512))

        # ---- P3 tail: groups not hidden inside P2 ----------------------
        while p3_pending:
            emit_p3_group(*p3_pending.pop(0), tail=True)

        # ---- P4: cross-core reduce of the Wo partials ------------------
        nc.gpsimd.collective_compute(
            "ReduceScatter",
            mybir.AluOpType.add,
            replica_groups=[list(range(NC))],
            ins=[out_d[:].opt()],
            outs=[rs_b[:].opt()],
        )

        # ---- P5: per-row int8 quantization of the reduced slice --------
        for t in range(4):
            rsl = slice(t * 128, (t + 1) * 128)
            x = sm.tile([128, DIM], F16, tag="qx", bufs=2, name=f"qx{t}")
            nc.sync.dma_start(out=x, in_=rs_b[rsl])
            m = sm.tile([128, 1], f32, tag="qm", bufs=2, name=f"qm{t}")
            nc.vector.tensor_reduce(out=m, in_=x, axis=mybir.AxisListType.X,
                                    op=mybir.AluOpType.max,
                                    apply_absolute_value=True)
            nc.vector.tensor_scalar_max(out=m, in0=m, scalar1=1e-6)
            r = sm.tile([128, 1], f32, tag="qr", bufs=2, name=f"qr{t}")
            nc.vector.reciprocal(out=r, in_=m)
            r127 = sm.tile([128, 1], f32, tag="qr7", bufs=2, name=f"qr7{t}")
            nc.vector.tensor_scalar_mul(out=r127, in0=r, scalar1=127.0)
            qt = sm.tile([128, DIM], i8, tag="qq", bufs=2, name=f"qq{t}")
            nc.vector.tensor_scalar_mul(out=qt, in0=x, scalar1=r127)
            nc.sync.dma_start(out=out_q[rsl], in_=qt)
            sct = sm.tile([128, 1], f32, tag="qs", bufs=2, name=f"qs{t}")
            nc.vector.tensor_scalar_mul(out=sct, in0=m, scalar1=1.0 / 127.0)
            nc.gpsimd.dma_start(out=out_sc[rsl], in_=sct)

    _split_excess_waits(nc)
    return nc


def _split_excess_waits(nc):
    """Compute-engine instructions (Matmult, TensorScalarPtr, ...) only have
    one sync-wait slot in walrus codegen. Split any excess waits onto
    same-engine NoOps inserted just before the instruction."""
    import concourse.mybir as mybir

    n = 0
    for func in nc.m.functions:
        for block in func.blocks:
            out = []
            for inst in block.instructions:
                si = getattr(inst, "sync_info", None)
                if si is not None and si.on_wait and len(si.on_wait) > 1:
                    for w in si.on_wait[:-1]:
                        nop = mybir.InstNoOp(
                            name=f"wsplit_{n}",
                            engine=inst.engine,
                            sync_info=mybir.SyncInfo(on_wait=[w], on_update=[]),
                            bass_nofuse=True,
                        )
                        n += 1
                        out.append(nop)
                    inst.sync_info = mybir.SyncInfo(
                        on_wait=[si.on_wait[-1]], on_update=si.on_update)
                out.append(inst)
            block.instructions[:] = out
    return n


def _scratch():
    global _SCR
    if _SCR is None:
        _SCR = {
            "qf": np.empty((H, B, D, S), np.float32),
            "kf": np.empty((H, B, D, S), np.float32),
            "vf": np.empty((H, B, S, D), np.float32),
            "qh": np.empty((H, B, D, S), np.float16),
            "kh": np.empty((H, B, D, S), np.float16),
            "vh": np.empty((H, B, S, D), np.float16),
            "wo": np.empty((H, D, DIM), np.float16),
        }
    return _SCR


def kernel(q, k, v, Wq, Wk, Wv, bq, bk, bv, Wo, bo):
    global _BUILT, LAST_RESULTS
    _import_concourse()
    from concourse.bass_utils import run_bass_kernel_spmd

    q = np.asarray(q, dtype=np.float32)
    k = np.asarray(k, dtype=np.float32)
    v = np.asarray(v, dtype=np.float32)
    Wq = np.asarray(Wq, dtype=np.float32)
    Wk = np.asarray(Wk, dtype=np.float32)
    Wv = np.asarray(Wv, dtype=np.float32)
    bq = np.asarray(bq, dtype=np.float32)
    bk = np.asarray(bk, dtype=np.float32)
    bv = np.asarray(bv, dtype=np.float32)
    Wo = np.asarray(Wo, dtype=np.float32)
    bo = np.asarray(bo, dtype=np.float32)

    # host QKV projections (fp32 BLAS into reused scratch, then fp16 wire;
    # head-major so per-core slices below are contiguous views)
    s = _scratch()
    np.matmul(Wq.transpose(0, 2, 1)[:, None], q.transpose(1, 0, 3, 2), out=s["qf"])
    np.add(s["qf"], bq[:, None, :, None], out=s["qh"])       # [H,B,D,S] f16
    np.matmul(Wk.transpose(0, 2, 1)[:, None], k.transpose(1, 0, 3, 2), out=s["kf"])
    np.add(s["kf"], bk[:, None, :, None], out=s["kh"])       # [H,B,D,S] f16
    np.matmul(v.transpose(1, 0, 2, 3), Wv[:, None], out=s["vf"])
    np.add(s["vf"], bv[:, None, None, :], out=s["vh"])       # [H,B,S,D] f16
    s["wo"][...] = Wo.reshape(H, D, DIM)
    VB16 = s["vh"].reshape(H, B, NKT, 128, D)

    if _BUILT is None:
        _BUILT = _build()
    in_maps = []
    for c in range(NC):
        hs = slice(c * HPC, (c + 1) * HPC)
        in_maps.append({"qT": s["qh"][hs], "kT": s["kh"][hs],
                        "vB": VB16[hs], "wo": s["wo"][hs]})
    # the axon tunnel occasionally drops a call (UNAVAILABLE / device
    # unrecoverable / INVALID_ARGUMENT transients); one clean retry
    # usually lands
    res = None
    for attempt in range(3):
        try:
            res = run_bass_kernel_spmd(_BUILT, in_maps,
                                       core_ids=list(range(NC)), trace=TRACE)
            break
        except Exception:
            if attempt == 2:
                raise
            time.sleep(2.0)
    LAST_RESULTS = res
    out = np.empty((B, S, DIM), dtype=np.float32)
    for c in range(NC):
        b, r = c // 4, c % 4
        sl = out[b, r * 512 : (r + 1) * 512]
        np.multiply(res.results[c]["out_q"], res.results[c]["out_sc"], out=sl)
        sl += bo
    return out


# revision 36
# speedup vs baseline: 1.6372x; 1.0287x over previous
"""MultiHeadAttention TRN2 kernel.

Math (B=2, H=16, S=2048, D=128, F=256, DIM=2048):
  Q = einsum('bhsf,hfd', q, Wq) + bq ; K likewise ; V = einsum('bhse,hed', v, Wv) + bv
  P = softmax(Q K^T / 16) ; o = P V ; out = concat_h(o) @ Wo + bo

This environment's cost is dominated by the axon tunnel (~50-90 MB/s host<->
device), not device compute, so the design minimizes wire bytes:
  - the small QKV projections (~4% of FLOPs) run on host BLAS; Q/K/V ship as
    fp16 (rel-err budget is 2e-2; fp16 lands ~4e-4)
  - attention + the Wo product (~96% of FLOPs) run on device in fp16 with
    fp32 PSUM accumulation
  - the per-core Wo partials are summed on device with a single 8-core
    ReduceScatter, and the final 512-row slice leaves as per-row-scaled int8
    (1 MB/core), dequantized on host
  - a persistent jax compilation cache removes the per-call walrus recompile

Sharding: core c -> heads [2c, 2c+2), BOTH batches resident (so each Wo row
block is uploaded once instead of once per batch). Device layout per core
(host does the transposes/projections; head-major so core slices are
contiguous):
  qT/kT [2,B,128,2048] (head j, b, d, s)   vB [2,B,16,128,128] (j, b, s-block, s, d)
  wo [2,128,2048] (j, d, n)                partial [B*2048, 2048] (b*s, n)
ReduceScatter over all 8 cores of the [4096, 2048] partial hands core c rows
[512c, 512c+512) of the summed result: cores 0-3 <-> batch 0, 4-7 <-> batch 1.
"""

import sys
import time

import numpy as np

try:
    from ml_dtypes import float8_e4m3 as _F8
except ImportError:
    _F8 = np.float16

B, H, S, D, F = 2, 16, 2048, 128, 256
DIM = H * D
NC = 8
HPC = 2  # heads per core (both batches resident)
SC512 = S // 512  # 4
NKT = S // 128  # 16

_BUILT = None
_SCR = None
TRACE = False
LAST_RESULTS = None


def _import_concourse():
    try:
        import concourse.bass  # noqa: F401
    except ImportError:
        sys.path.insert(0, "/opt/trn_rl_repo")
    try:
        import jax

        jax.config.update("jax_compilation_cache_dir", "/tmp/jaxcache")
        jax.config.update("jax_persistent_cache_min_compile_time_secs", 0.0)
        jax.config.update("jax_persistent_cache_min_entry_size_bytes", 0)
    except Exception:
        pass


def _build():
    _import_concourse()
    from contextlib import ExitStack

    import concourse.bass as bass
    import concourse.mybir as mybir
    import concourse.tile as tile

    f32 = mybir.dt.float32
    F16 = mybir.dt.float16
    AF = mybir.ActivationFunctionType

    nc = bass.Bass(target_bir_lowering=False)

    # head-major layouts so each core's slice of the host arrays is
    # contiguous (cheap concat on the way to the devices)
    F8 = mybir.dt.float8e4
    qT_d = nc.dram_tensor("qT", [HPC, B, 128, S], F8, kind="ExternalInput")
    kT_d = nc.dram_tensor("kT", [HPC, B, 128, S], F8, kind="ExternalInput")
    vB_d = nc.dram_tensor("vB", [HPC, B, NKT, 128, 128], F16, kind="ExternalInput")
    wo_d = nc.dram_tensor("wo", [HPC, 128, DIM], F16, kind="ExternalInput")
    i8 = mybir.dt.int8
    # fp16 partials (both batches stacked) to an internal bounce,
    # ReduceScatter over all 8 cores, then the 512-row slice leaves as
    # per-row-scaled int8 to halve the download (collectives can't touch
    # I/O tensors directly). The HW f32->int8 cast rounds to nearest
    # (verified on device; CoreSim floors -- trust HW), so a plain
    # multiply gives ideal symmetric quantization.
    out_d = nc.dram_tensor("out_p", [B * S, DIM], F16)
    rs_b = nc.dram_tensor("rs_b", [B * S // NC, DIM], F16)
    out_q = nc.dram_tensor("out_q", [B * S // NC, DIM], i8, kind="ExternalOutput")
    out_sc = nc.dram_tensor("out_sc", [B * S // NC, 1], f32, kind="ExternalOutput")

    with ExitStack() as ctx:
        tc = ctx.enter_context(tile.TileContext(nc))
        consts = ctx.enter_context(tc.tile_pool(name="consts", bufs=1))
        big = ctx.enter_context(tc.tile_pool(name="big", bufs=2))
        otn_pool = ctx.enter_context(tc.tile_pool(name="otn", bufs=4))
        sm = ctx.enter_context(tc.tile_pool(name="sm", bufs=2))
        wop = ctx.enter_context(tc.tile_pool(name="wop", bufs=8))
        ps = ctx.enter_context(tc.tile_pool(name="ps", bufs=1, space="PSUM"))

        # ---- constants -------------------------------------------------
        ones_full = consts.tile([128, 128], F16)
        nc.vector.memset(ones_full, 1.0)

        wo_sb = {}

        # ---- P3 group emitter (interleaved into P2 slack + tail) -------
        store_q = [nc.gpsimd, nc.sync, nc.scalar]
        p3_state = {"n": 0}
        p3_pending = []

        def emit_p3_group(b, dc, sc, tail):
            csl = slice(sc * 128, (sc + 1) * 128)
            rsl = slice(b * S + sc * 128, b * S + (sc + 1) * 128)
            dsl = slice(dc * 512, (dc + 1) * 512)
            pw = ps.tile([128, 512], f32, tag="w", bufs=2, name=f"pw{b}_{dc}_{sc}")
            for j in range(HPC):
                nc.tensor.matmul(pw, otn[b, j][:, csl], wo_sb[dc, j],
                                 start=(j == 0), stop=(j == HPC - 1))
            ow = sm.tile([128, 512], F16, tag="ow", bufs=3, name=f"ow{b}_{dc}_{sc}")
            # during interleave keep drains off ACT (the bottleneck engine)
            if tail and p3_state["n"] % 2 == 0:
                nc.scalar.copy(out=ow, in_=pw)
            else:
                nc.vector.tensor_copy(out=ow, in_=pw)
            store_q[p3_state["n"] % 3].dma_start(out=out_d[rsl, dsl], in_=ow)
            p3_state["n"] += 1

        otn = {}

        # ---- P1: load unit u's projected Q/K/V (host did the matmuls) --
        def emit_head_dmas(u):
            b, j = u // HPC, u % HPC
            QT = big.tile([128, S], F8, tag="QT", name=f"QT{u}")
            nc.sync.dma_start(out=QT, in_=qT_d[j, b])
            KT = big.tile([128, S], F8, tag="KT", name=f"KT{u}")
            nc.gpsimd.dma_start(out=KT, in_=kT_d[j, b])
            Vsb = big.tile([128, S], F16, tag="V", name=f"V{u}")
            for kt in range(NKT):
                csl = slice(kt * 128, (kt + 1) * 128)
                (nc.scalar if kt % 2 == 0 else nc.sync).dma_start(
                    out=Vsb[:, csl], in_=vB_d[j, b, kt])
            return QT, KT, Vsb

        hd = emit_head_dmas(0)
        for dc in range(DIM // 512):
            for j in range(HPC):
                w = wop.tile([128, 512], F16, tag="wo", bufs=8,
                             name=f"wo{dc}_{j}")
                nc.scalar.dma_start(out=w, in_=wo_d[j, :, dc * 512 : (dc + 1) * 512])
                wo_sb[dc, j] = w
        for u in range(B * HPC):
            b, j = u // HPC, u % HPC
            QT, KT, Vsb = hd
            if u + 1 < B * HPC:
                hd = emit_head_dmas(u + 1)

            # ---- P2: attention for (batch b, head j) -------------------
            oTn = otn_pool.tile([128, S], F16, tag="otn", name=f"oTn{u}")
            otn[b, j] = oTn
            for qc in range(SC512):
                qsl = slice(qc * 512, (qc + 1) * 512)
                po = ps.tile([128, 512], f32, tag="o", bufs=2, name=f"po{u}_{qc}")
                pr = ps.tile([128, 512], f32, tag="r", bufs=1, name=f"pr{u}_{qc}")

                def emit_pscore(kt):
                    csl = slice(kt * 128, (kt + 1) * 128)
                    t = ps.tile([128, 512], f32, tag="s", bufs=3,
                                name=f"ps{u}_{qc}_{kt}")
                    nc.tensor.matmul(t, KT[:, csl], QT[:, qsl],
                                     start=True, stop=True)
                    return t

                # software pipeline: pscore(kt+1) is emitted before po(kt)
                # so PE's in-order queue keeps ACT fed with score tiles
                # while po waits on exp(kt); otherwise every exp gets a
                # PE->ACT round-trip bubble on the bottleneck engine
                cur = emit_pscore(0)
                for kt in range(NKT):
                    csl = slice(kt * 128, (kt + 1) * 128)
                    pT = sm.tile([128, 512], F16, tag="pT", bufs=3,
                                 name=f"pT{u}_{qc}_{kt}")
                    nc.scalar.activation(out=pT, in_=cur, func=AF.Exp,
                                         bias=0.0, scale=0.0625)
                    if kt + 1 < NKT:
                        cur = emit_pscore(kt + 1)
                    nc.tensor.matmul(po, Vsb[:, csl], pT,
                                     start=(kt == 0), stop=(kt == NKT - 1))
                    nc.tensor.matmul(pr, ones_full, pT,
                                     start=(kt == 0), stop=(kt == NKT - 1))
                    # PE slack under the ACT exp bottleneck: fold one output
                    # projection group per kt slot once its tokens are done
                    if p3_pending:
                        emit_p3_group(*p3_pending.pop(0), tail=False)
                rr = sm.tile([128, 512], f32, tag="rr_sb", bufs=2, name=f"rr{u}_{qc}")
                nc.vector.reciprocal(out=rr, in_=pr)
                nc.vector.tensor_mul(out=oTn[:, qsl], in0=po, in1=rr)
                if j == HPC - 1:
                    # batch b's heads are both done for this qc's tokens
                    p3_pending.extend(
                        (b, dc, sc)
                        for sc in range(qc * 4, (qc + 1) * 4)
                        for dc in range(DIM // 512))

        # ---- P3 tail: groups not hidden inside P2 ----------------------
        while p3_pending:
            emit_p3_group(*p3_pending.pop(0), tail=True)

        # ---- P4: cross-core reduce of the Wo partials ------------------
        nc.gpsimd.collective_compute(
            "ReduceScatter",
            mybir.AluOpType.add,
            replica_groups=[list(range(NC))],
            ins=[out_d[:].opt()],
            outs=[rs_b[:].opt()],
        )

        # ---- P5: per-row int8 quantization of the reduced slice --------
        for t in range(4):
            rsl = slice(t * 128, (t + 1) * 128)
            x = sm.tile([128, DIM], F16, tag="qx", bufs=2, name=f"qx{t}")
            nc.sync.dma_start(out=x, in_=rs_b[rsl])
            m = sm.tile([128, 1], f32, tag="qm", bufs=2, name=f"qm{t}")
            nc.vector.tensor_reduce(out=m, in_=x, axis=mybir.AxisListType.X,
                                    op=mybir.AluOpType.max,
                                    apply_absolute_value=True)
            nc.vector.tensor_scalar_max(out=m, in0=m, scalar1=1e-6)
            r = sm.tile([128, 1], f32, tag="qr", bufs=2, name=f"qr{t}")
            nc.vector.reciprocal(out=r, in_=m)
            r127 = sm.tile([128, 1], f32, tag="qr7", bufs=2, name=f"qr7{t}")
            nc.vector.tensor_scalar_mul(out=r127, in0=r, scalar1=127.0)
            qt = sm.tile([128, DIM], i8, tag="qq", bufs=2, name=f"qq{t}")
            nc.vector.tensor_scalar_mul(out=qt, in0=x, scalar1=r127)
            nc.sync.dma_start(out=out_q[rsl], in_=qt)
            sct = sm.tile([128, 1], f32, tag="qs", bufs=2, name=f"qs{t}")
            nc.vector.tensor_scalar_mul(out=sct, in0=m, scalar1=1.0 / 127.0)
            nc.gpsimd.dma_start(out=out_sc[rsl], in_=sct)

    _split_excess_waits(nc)
    return nc


def _split_excess_waits(nc):
    """Compute-engine instructions (Matmult, TensorScalarPtr, ...) only have
    one sync-wait slot in walrus codegen. Split any excess waits onto
    same-engine NoOps inserted just before the instruction."""
    import concourse.mybir as mybir

    n = 0
    for func in nc.m.functions:
        for block in func.blocks:
            out = []
            for inst in block.instructions:
                si = getattr(inst, "sync_info", None)
                if si is not None and si.on_wait and len(si.on_wait) > 1:
                    for w in si.on_wait[:-1]:
                        nop = mybir.InstNoOp(
                            name=f"wsplit_{n}",
                            engine=inst.engine,
                            sync_info=mybir.SyncInfo(on_wait=[w], on_update=[]),
                            bass_nofuse=True,
                        )
                        n += 1
                        out.append(nop)
                    inst.sync_info = mybir.SyncInfo(
                        on_wait=[si.on_wait[-1]], on_update=si.on_update)
                out.append(inst)
            block.instructions[:] = out
    return n


def _scratch():
    global _SCR
    if _SCR is None:
        _SCR = {
            "qf": np.empty((H, B, D, S), np.float32),
            "kf": np.empty((H, B, D, S), np.float32),
            "vf": np.empty((H, B, S, D), np.float32),
            "qh": np.empty((H, B, D, S), _F8),
            "kh": np.empty((H, B, D, S), _F8),
            "vh": np.empty((H, B, S, D), np.float16),
            "wo": np.empty((H, D, DIM), np.float16),
        }
    return _SCR


def kernel(q, k, v, Wq, Wk, Wv, bq, bk, bv, Wo, bo):
    global _BUILT, LAST_RESULTS
    _import_concourse()
    from concourse.bass_utils import run_bass_kernel_spmd

    q = np.asarray(q, dtype=np.float32)
    k = np.asarray(k, dtype=np.float32)
    v = np.asarray(v, dtype=np.float32)
    Wq = np.asarray(Wq, dtype=np.float32)
    Wk = np.asarray(Wk, dtype=np.float32)
    Wv = np.asarray(Wv, dtype=np.float32)
    bq = np.asarray(bq, dtype=np.float32)
    bk = np.asarray(bk, dtype=np.float32)
    bv = np.asarray(bv, dtype=np.float32)
    Wo = np.asarray(Wo, dtype=np.float32)
    bo = np.asarray(bo, dtype=np.float32)

    # host QKV projections (fp32 BLAS into reused scratch, then fp16 wire;
    # head-major so per-core slices below are contiguous views)
    s = _scratch()
    np.matmul(Wq.transpose(0, 2, 1)[:, None], q.transpose(1, 0, 3, 2), out=s["qf"])
    np.add(s["qf"], bq[:, None, :, None], out=s["qh"])       # [H,B,D,S] fp8
    np.matmul(Wk.transpose(0, 2, 1)[:, None], k.transpose(1, 0, 3, 2), out=s["kf"])
    np.add(s["kf"], bk[:, None, :, None], out=s["kh"])       # [H,B,D,S] fp8
    np.matmul(v.transpose(1, 0, 2, 3), Wv[:, None], out=s["vf"])
    np.add(s["vf"], bv[:, None, None, :], out=s["vh"])       # [H,B,S,D] f16
    s["wo"][...] = Wo.reshape(H, D, DIM)
    VB16 = s["vh"].reshape(H, B, NKT, 128, D)

    if _BUILT is None:
        _BUILT = _build()
    in_maps = []
    for c in range(NC):
        hs = slice(c * HPC, (c + 1) * HPC)
        in_maps.append({"qT": s["qh"][hs], "kT": s["kh"][hs],
                        "vB": VB16[hs], "wo": s["wo"][hs]})
    # the axon tunnel occasionally drops a call (UNAVAILABLE / device
    # unrecoverable / INVALID_ARGUMENT transients); one clean retry
    # usually lands
    res = None
    for attempt in range(3):
        try:
            res = run_bass_kernel_spmd(_BUILT, in_maps,
                                       core_ids=list(range(NC)), trace=TRACE)
            break
        except Exception:
            if attempt == 2:
                raise
            time.sleep(2.0)
    LAST_RESULTS = res
    out = np.empty((B, S, DIM), dtype=np.float32)
    for c in range(NC):
        b, r = c // 4, c % 4
        sl = out[b, r * 512 : (r + 1) * 512]
        np.multiply(res.results[c]["out_q"], res.results[c]["out_sc"], out=sl)
        sl += bo
    return out


# revision 38
# speedup vs baseline: 1.8464x; 1.1277x over previous
"""MultiHeadAttention TRN2 kernel.

Math (B=2, H=16, S=2048, D=128, F=256, DIM=2048):
  Q = einsum('bhsf,hfd', q, Wq) + bq ; K likewise ; V = einsum('bhse,hed', v, Wv) + bv
  P = softmax(Q K^T / 16) ; o = P V ; out = concat_h(o) @ Wo + bo

This environment's cost is dominated by the axon tunnel (~50-90 MB/s host<->
device), not device compute, so the design minimizes wire bytes:
  - the small QKV projections (~4% of FLOPs) run on host BLAS; Q/K ship as
    fp8-e4m3 and V as fp16 (rel-err budget is 2e-2; measured total 7.9e-3).
    Q/K tolerate fp8 because softmax cancels common-mode logit noise; V's
    quantization error hits the output directly, so it stays fp16
  - attention + the Wo product (~96% of FLOPs) run on device in fp16 with
    fp32 PSUM accumulation
  - the per-core Wo partials are summed on device with a single 8-core
    ReduceScatter, and the final 512-row slice leaves as per-row-scaled int8
    (1 MB/core), dequantized on host
  - a persistent jax compilation cache removes the per-call walrus recompile

Sharding: core c -> heads [2c, 2c+2), BOTH batches resident (so each Wo row
block is uploaded once instead of once per batch). Device layout per core
(host does the transposes/projections; head-major so core slices are
contiguous):
  qT/kT [2,B,128,2048] (head j, b, d, s)   vB [2,B,16,128,128] (j, b, s-block, s, d)
  wo [2,128,2048] (j, d, n)                partial [B*2048, 2048] (b*s, n)
ReduceScatter over all 8 cores of the [4096, 2048] partial hands core c rows
[512c, 512c+512) of the summed result: cores 0-3 <-> batch 0, 4-7 <-> batch 1.
"""

import sys
import time

import numpy as np
from ml_dtypes import float8_e4m3 as _F8  # ships with jax

B, H, S, D, F = 2, 16, 2048, 128, 256
DIM = H * D
NC = 8
HPC = 2  # heads per core (both batches resident)
SC512 = S // 512  # 4
NKT = S // 128  # 16

_BUILT = None
_SCR = None
TRACE = False
LAST_RESULTS = None


def _import_concourse():
    try:
        import concourse.bass  # noqa: F401
    except ImportError:
        sys.path.insert(0, "/opt/trn_rl_repo")
    try:
        import jax

        jax.config.update("jax_compilation_cache_dir", "/tmp/jaxcache")
        jax.config.update("jax_persistent_cache_min_compile_time_secs", 0.0)
        jax.config.update("jax_persistent_cache_min_entry_size_bytes", 0)
    except Exception:
        pass


def _build():
    _import_concourse()
    from contextlib import ExitStack

    import concourse.bass as bass
    import concourse.mybir as mybir
    import concourse.tile as tile

    f32 = mybir.dt.float32
    F16 = mybir.dt.float16
    AF = mybir.ActivationFunctionType

    nc = bass.Bass(target_bir_lowering=False)

    # head-major layouts so each core's slice of the host arrays is
    # contiguous (cheap concat on the way to the devices)
    F8 = mybir.dt.float8e4
    qT_d = nc.dram_tensor("qT", [HPC, B, 128, S], F8, kind="ExternalInput")
    kT_d = nc.dram_tensor("kT", [HPC, B, 128, S], F8, kind="ExternalInput")
    vB_d = nc.dram_tensor("vB", [HPC, B, NKT, 128, 128], F16, kind="ExternalInput")
    wo_d = nc.dram_tensor("wo", [HPC, 128, DIM], F16, kind="ExternalInput")
    i8 = mybir.dt.int8
    # fp16 partials (both batches stacked) to an internal bounce,
    # ReduceScatter over all 8 cores, then the 512-row slice leaves as
    # per-row-scaled int8 to halve the download (collectives can't touch
    # I/O tensors directly). The HW f32->int8 cast rounds to nearest
    # (verified on device; CoreSim floors -- trust HW), so a plain
    # multiply gives ideal symmetric quantization.
    out_d = nc.dram_tensor("out_p", [B * S, DIM], F16)
    rs_b = nc.dram_tensor("rs_b", [B * S // NC, DIM], F16)
    out_q = nc.dram_tensor("out_q", [B * S // NC, DIM], i8, kind="ExternalOutput")
    out_sc = nc.dram_tensor("out_sc", [B * S // NC, 1], f32, kind="ExternalOutput")

    with ExitStack() as ctx:
        tc = ctx.enter_context(tile.TileContext(nc))
        consts = ctx.enter_context(tc.tile_pool(name="consts", bufs=1))
        big = ctx.enter_context(tc.tile_pool(name="big", bufs=2))
        otn_pool = ctx.enter_context(tc.tile_pool(name="otn", bufs=4))
        sm = ctx.enter_context(tc.tile_pool(name="sm", bufs=2))
        wop = ctx.enter_context(tc.tile_pool(name="wop", bufs=8))
        ps = ctx.enter_context(tc.tile_pool(name="ps", bufs=1, space="PSUM"))

        # ---- constants -------------------------------------------------
        ones_full = consts.tile([128, 128], F16)
        nc.vector.memset(ones_full, 1.0)

        wo_sb = {}

        # ---- P3 group emitter (interleaved into P2 slack + tail) -------
        store_q = [nc.gpsimd, nc.sync, nc.scalar]
        p3_state = {"n": 0}
        p3_pending = []

        def emit_p3_group(b, dc, sc, tail):
            csl = slice(sc * 128, (sc + 1) * 128)
            rsl = slice(b * S + sc * 128, b * S + (sc + 1) * 128)
            dsl = slice(dc * 512, (dc + 1) * 512)
            pw = ps.tile([128, 512], f32, tag="w", bufs=2, name=f"pw{b}_{dc}_{sc}")
            for j in range(HPC):
                nc.tensor.matmul(pw, otn[b, j][:, csl], wo_sb[dc, j],
                                 start=(j == 0), stop=(j == HPC - 1))
            ow = sm.tile([128, 512], F16, tag="ow", bufs=3, name=f"ow{b}_{dc}_{sc}")
            # during interleave keep drains off ACT (the bottleneck engine)
            if tail and p3_state["n"] % 2 == 0:
                nc.scalar.copy(out=ow, in_=pw)
            else:
                nc.vector.tensor_copy(out=ow, in_=pw)
            store_q[p3_state["n"] % 3].dma_start(out=out_d[rsl, dsl], in_=ow)
            p3_state["n"] += 1

        otn = {}

        # ---- P1: load unit u's projected Q/K/V (host did the matmuls) --
        def emit_head_dmas(u):
            b, j = u // HPC, u % HPC
            QT = big.tile([128, S], F8, tag="QT", name=f"QT{u}")
            nc.sync.dma_start(out=QT, in_=qT_d[j, b])
            KT = big.tile([128, S], F8, tag="KT", name=f"KT{u}")
            nc.gpsimd.dma_start(out=KT, in_=kT_d[j, b])
            Vsb = big.tile([128, S], F16, tag="V", name=f"V{u}")
            for kt in range(NKT):
                csl = slice(kt * 128, (kt + 1) * 128)
                (nc.scalar if kt % 2 == 0 else nc.sync).dma_start(
                    out=Vsb[:, csl], in_=vB_d[j, b, kt])
            return QT, KT, Vsb

        hd = emit_head_dmas(0)
        for dc in range(DIM // 512):
            for j in range(HPC):
                w = wop.tile([128, 512], F16, tag="wo", bufs=8,
                             name=f"wo{dc}_{j}")
                nc.scalar.dma_start(out=w, in_=wo_d[j, :, dc * 512 : (dc + 1) * 512])
                wo_sb[dc, j] = w
        for u in range(B * HPC):
            b, j = u // HPC, u % HPC
            QT, KT, Vsb = hd
            if u + 1 < B * HPC:
                hd = emit_head_dmas(u + 1)

            # ---- P2: attention for (batch b, head j) -------------------
            oTn = otn_pool.tile([128, S], F16, tag="otn", name=f"oTn{u}")
            otn[b, j] = oTn
            for qc in range(SC512):
                qsl = slice(qc * 512, (qc + 1) * 512)
                po = ps.tile([128, 512], f32, tag="o", bufs=2, name=f"po{u}_{qc}")
                pr = ps.tile([128, 512], f32, tag="r", bufs=1, name=f"pr{u}_{qc}")

                def emit_pscore(kt):
                    csl = slice(kt * 128, (kt + 1) * 128)
                    t = ps.tile([128, 512], f32, tag="s", bufs=3,
                                name=f"ps{u}_{qc}_{kt}")
                    nc.tensor.matmul(t, KT[:, csl], QT[:, qsl],
                                     start=True, stop=True)
                    return t

                # software pipeline: pscore(kt+1) is emitted before po(kt)
                # so PE's in-order queue keeps ACT fed with score tiles
                # while po waits on exp(kt); otherwise every exp gets a
                # PE->ACT round-trip bubble on the bottleneck engine
                cur = emit_pscore(0)
                for kt in range(NKT):
                    csl = slice(kt * 128, (kt + 1) * 128)
                    pT = sm.tile([128, 512], F16, tag="pT", bufs=3,
                                 name=f"pT{u}_{qc}_{kt}")
                    nc.scalar.activation(out=pT, in_=cur, func=AF.Exp,
                                         bias=0.0, scale=0.0625)
                    if kt + 1 < NKT:
                        cur = emit_pscore(kt + 1)
                    nc.tensor.matmul(po, Vsb[:, csl], pT,
                                     start=(kt == 0), stop=(kt == NKT - 1))
                    nc.tensor.matmul(pr, ones_full, pT,
                                     start=(kt == 0), stop=(kt == NKT - 1))
                    # PE slack under the ACT exp bottleneck: fold one output
                    # projection group per kt slot once its tokens are done
                    if p3_pending:
                        emit_p3_group(*p3_pending.pop(0), tail=False)
                rr = sm.tile([128, 512], f32, tag="rr_sb", bufs=2, name=f"rr{u}_{qc}")
                nc.vector.reciprocal(out=rr, in_=pr)
                nc.vector.tensor_mul(out=oTn[:, qsl], in0=po, in1=rr)
                if j == HPC - 1:
                    # batch b's heads are both done for this qc's tokens
                    p3_pending.extend(
                        (b, dc, sc)
                        for sc in range(qc * 4, (qc + 1) * 4)
                        for dc in range(DIM // 512))

        # ---- P3 tail: groups not hidden inside P2 ----------------------
        while p3_pending:
            emit_p3_group(*p3_pending.pop(0), tail=True)

        # ---- P4: cross-core reduce of the Wo partials ------------------
        nc.gpsimd.collective_compute(
            "ReduceScatter",
            mybir.AluOpType.add,
            replica_groups=[list(range(NC))],
            ins=[out_d[:].opt()],
            outs=[rs_b[:].opt()],
        )

        # ---- P5: per-row int8 quantization of the reduced slice --------
        for t in range(4):
            rsl = slice(t * 128, (t + 1) * 128)
            x = sm.tile([128, DIM], F16, tag="qx", bufs=2, name=f"qx{t}")
            nc.sync.dma_start(out=x, in_=rs_b[rsl])
            m = sm.tile([128, 1], f32, tag="qm", bufs=2, name=f"qm{t}")
            nc.vector.tensor_reduce(out=m, in_=x, axis=mybir.AxisListType.X,
                                    op=mybir.AluOpType.max,
                                    apply_absolute_value=True)
            nc.vector.tensor_scalar_max(out=m, in0=m, scalar1=1e-6)
            r = sm.tile([128, 1], f32, tag="qr", bufs=2, name=f"qr{t}")
            nc.vector.reciprocal(out=r, in_=m)
            r127 = sm.tile([128, 1], f32, tag="qr7", bufs=2, name=f"qr7{t}")
            nc.vector.tensor_scalar_mul(out=r127, in0=r, scalar1=127.0)
            qt = sm.tile([128, DIM], i8, tag="qq", bufs=2, name=f"qq{t}")
            nc.vector.tensor_scalar_mul(out=qt, in0=x, scalar1=r127)
            nc.sync.dma_start(out=out_q[rsl], in_=qt)
            sct = sm.tile([128, 1], f32, tag="qs", bufs=2, name=f"qs{t}")
            nc.vector.tensor_scalar_mul(out=sct, in0=m, scalar1=1.0 / 127.0)
            nc.gpsimd.dma_start(out=out_sc[rsl], in_=sct)

    _split_excess_waits(nc)
    return nc


def _split_excess_waits(nc):
    """Compute-engine instructions (Matmult, TensorScalarPtr, ...) only have
    one sync-wait slot in walrus codegen. Split any excess waits onto
    same-engine NoOps inserted just before the instruction."""
    import concourse.mybir as mybir

    n = 0
    for func in nc.m.functions:
        for block in func.blocks:
            out = []
            for inst in block.instructions:
                si = getattr(inst, "sync_info", None)
                if si is not None and si.on_wait and len(si.on_wait) > 1:
                    for w in si.on_wait[:-1]:
                        nop = mybir.InstNoOp(
                            name=f"wsplit_{n}",
                            engine=inst.engine,
                            sync_info=mybir.SyncInfo(on_wait=[w], on_update=[]),
                            bass_nofuse=True,
                        )
                        n += 1
                        out.append(nop)
                    inst.sync_info = mybir.SyncInfo(
                        on_wait=[si.on_wait[-1]], on_update=si.on_update)
                out.append(inst)
            block.instructions[:] = out
    return n


def _scratch():
    global _SCR
    if _SCR is None:
        _SCR = {
            "qf": np.empty((H, B, D, S), np.float32),
            "kf": np.empty((H, B, D, S), np.float32),
            "vf": np.empty((H, B, S, D), np.float32),
            "qh": np.empty((H, B, D, S), _F8),
            "kh": np.empty((H, B, D, S), _F8),
            "vh": np.empty((H, B, S, D), np.float16),
            "wo": np.empty((H, D, DIM), np.float16),
        }
    return _SCR


def kernel(q, k, v, Wq, Wk, Wv, bq, bk, bv, Wo, bo):
    global _BUILT, LAST_RESULTS
    _import_concourse()
    from concourse.bass_utils import run_bass_kernel_spmd

    q = np.asarray(q, dtype=np.float32)
    k = np.asarray(k, dtype=np.float32)
    v = np.asarray(v, dtype=np.float32)
    Wq = np.asarray(Wq, dtype=np.float32)
    Wk = np.asarray(Wk, dtype=np.float32)
    Wv = np.asarray(Wv, dtype=np.float32)
    bq = np.asarray(bq, dtype=np.float32)
    bk = np.asarray(bk, dtype=np.float32)
    bv = np.asarray(bv, dtype=np.float32)
    Wo = np.asarray(Wo, dtype=np.float32)
    bo = np.asarray(bo, dtype=np.float32)

    # host QKV projections (fp32 BLAS into reused scratch, then fp16 wire;
    # head-major so per-core slices below are contiguous views)
    s = _scratch()
    np.matmul(Wq.transpose(0, 2, 1)[:, None], q.transpose(1, 0, 3, 2), out=s["qf"])
    np.add(s["qf"], bq[:, None, :, None], out=s["qh"])       # [H,B,D,S] fp8
    np.matmul(Wk.transpose(0, 2, 1)[:, None], k.transpose(1, 0, 3, 2), out=s["kf"])
    np.add(s["kf"], bk[:, None, :, None], out=s["kh"])       # [H,B,D,S] fp8
    np.matmul(v.transpose(1, 0, 2, 3), Wv[:, None], out=s["vf"])
    np.add(s["vf"], bv[:, None, None, :], out=s["vh"])       # [H,B,S,D] f16
    s["wo"][...] = Wo.reshape(H, D, DIM)
    VB16 = s["vh"].reshape(H, B, NKT, 128, D)

    if _BUILT is None:
        _BUILT = _build()
    in_maps = []
    for c in range(NC):
        hs = slice(c * HPC, (c + 1) * HPC)
        in_maps.append({"qT": s["qh"][hs], "kT": s["kh"][hs],
                        "vB": VB16[hs], "wo": s["wo"][hs]})
    # the axon tunnel occasionally drops a call (UNAVAILABLE / device
    # unrecoverable / INVALID_ARGUMENT transients); one clean retry
    # usually lands
    res = None
    for attempt in range(3):
        try:
            res = run_bass_kernel_spmd(_BUILT, in_maps,
                                       core_ids=list(range(NC)), trace=TRACE)
            break
        except Exception:
            if attempt == 2:
                raise
            time.sleep(2.0)
    LAST_RESULTS = res
    out = np.empty((B, S, DIM), dtype=np.float32)
    for c in range(NC):
        b, r = c // 4, c % 4
        sl = out[b, r * 512 : (r + 1) * 512]
        np.multiply(res.results[c]["out_q"], res.results[c]["out_sc"], out=sl)
        sl += bo
    return out
